# revision 1
# baseline (speedup 1.0000x reference)
"""TRN2 Bass kernel: 4096x4096 fp32 'valid' cross-correlation with a 15x15
kernel (+ scalar bias), sharded row-wise across 8 NeuronCores.

Formulation (per core, per output row-tile of 128 rows, "corner" scheme):
  out[i, j] = sum_dj sum_di w[di, dj] * x[i + di, j + dj]
For each kernel column dj, the contraction over di is a banded-Toeplitz
matmul over input rows: stationary lhsT[k, m] = w[k - m, dj] (15-diagonal
band), moving rhs = x rows with a free-dim column offset of dj. The 15
dj-matmuls accumulate in one PSUM bank; the 14 seam rows per tile are
completed by two extra matmuls over replicated shifted copies (r) of the
14 rows below the tile's K-block, so each 128-row tile costs exactly
ceil(142*15/128) = 17 matmul streams — the PE coverage floor.

Matmuls run in bfloat16 (stationary and moving): 1 col/cycle on the PE
with the per-matmul weight load hidden (fast weight load path; fp32r
self-loads serialize ~90ns/matmul instead), at ~2.4e-3 relative error
(gate is 2e-2). The output is evacuated PSUM->SBUF as bf16 (halves y DMA
traffic) and the host casts back to fp32.

Each core gets 512 padded output rows (input slice of 526 rows including
the 14-row halo); the host pads x to 4110 rows and drops the 14 garbage
output rows at the end.
"""

import os
import sys

for _p in ("/opt/trn_rl_repo",):
    if os.path.isdir(_p) and _p not in sys.path:
        sys.path.insert(0, _p)

import json

import numpy as np

import concourse.bass as bass
import concourse.tile as tile
from concourse import mybir
from concourse.bass_utils import run_bass_kernel_spmd

# ---------------------------------------------------------------------------
# Workaround: the installed walrus_driver rejects instructions carrying more
# than one sync wait ("Too many sync wait commands"). TileContext's kernel-tail
# drain carries one wait per outstanding semaphore. Splitting each extra wait
# into its own single-wait EventSemaphore on the same engine right before the
# original instruction is semantically identical (same-engine program order;
# semaphores are monotone).
# ---------------------------------------------------------------------------
_orig_to_json_bytes = bass.Bass.to_json_bytes


def _split_multi_waits(bir: dict) -> dict:
    n = 0
    for fn in bir.get("functions", []):
        for bb in fn.get("blocks", []):
            insts = bb.get("instructions")
            if not insts:
                continue
            out = []
            for inst in insts:
                si = inst.get("sync_info") or {}
                waits = si.get("on_wait") or []
                if len(waits) > 1:
                    for w in waits[:-1]:
                        n += 1
                        out.append(
                            {
                                "debug": inst.get("debug", 0),
                                "engine": inst["engine"],
                                "ins": [],
                                "name": f"{inst['name']}-waitsplit{n}",
                                "opcode": "EventSemaphore",
                                "outs": [],
                                "sync_info": {"on_update": [], "on_wait": [w]},
                            }
                        )
                    si["on_wait"] = [waits[-1]]
                out.append(inst)
            bb["instructions"] = out
    return bir


def _patched_to_json_bytes(self, *args, **kwargs):
    raw = _orig_to_json_bytes(self, *args, **kwargs)
    return json.dumps(_split_multi_waits(json.loads(raw))).encode()


bass.Bass.to_json_bytes = _patched_to_json_bytes

# ---------------------------------------------------------------------------

H = W = 4096
KS = 15
OUT_H = H - KS + 1  # 4082
OUT_W = W - KS + 1  # 4082
N_CORES = 8
ROWS_PER_CORE = 512  # padded output rows per core (8 * 512 = 4096 >= 4082)
IN_ROWS = ROWS_PER_CORE + KS - 1  # 526

# Output row-tiles per core: M <= 114 so the band (M + 14) fits in K <= 128.
M_TILES = [(0, 114), (114, 114), (228, 114), (342, 114), (456, 56)]
# Corner scheme: 4 tiles of 128 rows; the 14 seam rows per tile are completed
# by two extra matmuls contracting (dj, k') pairs over replicated shifted
# copies of the 14 rows below the tile's K-block.
M_TILES_CORNER = [(0, 128), (128, 128), (256, 128), (384, 128)]
CORNER_SPLIT = 9  # dj 0..8 -> corner MM a (K=126), dj 9..14 -> MM b (K=84)
# Output column tiles: N <= 512 (one fp32 PSUM bank).
N_TILES = [(c, min(512, OUT_W - c)) for c in range(0, OUT_W, 512)]

F32R = mybir.dt.float32r
F32 = mybir.dt.float32
BF16 = mybir.dt.bfloat16


def build_program(
    bias_val: float,
    repeat: int = 1,
    loop_repeat: int = 1,
    loop_order: str = "c_dj",
    evacuate: bool = True,
    psum_bufs: int = 4,
    same_stationary: bool = False,
    pure_mm: bool = False,
    x_bufs: int = 2,
    y_per_ctile: bool = False,
    split_dma: int = 1,
    evac_any: bool = False,
    r_split: int = 1,
    in_dtype=F32R,
    psum_dma: bool = False,
    djc: bool = False,
    no_corner_mm: bool = False,
    pure_mm_m128: bool = False,
    x_redma: bool = False,
    y_bf16: bool = False,
    n_wide: int = 512,
    r_bufs: int = 2,
) -> bass.Bass:
    nc = bass.Bass()
    x_d = nc.dram_tensor("x", [IN_ROWS, W], in_dtype, kind="ExternalInput")
    t_d = nc.dram_tensor("t", [128, KS, 128], in_dtype, kind="ExternalInput")
    t2_d = nc.dram_tensor("t2", [14 * KS, 128], in_dtype, kind="ExternalInput")
    r_d = nc.dram_tensor(
        "r", [len(M_TILES_CORNER), 14 * KS, OUT_W], in_dtype, kind="ExternalInput"
    )
    y_dt = BF16 if y_bf16 else F32
    y_d = nc.dram_tensor("y", [ROWS_PER_CORE, OUT_W], y_dt, kind="ExternalOutput")

    with tile.TileContext(nc) as tc:
        with (
            tc.tile_pool(name="tconst", bufs=1) as tpool,
            tc.tile_pool(name="xin", bufs=x_bufs) as xpool,
            tc.tile_pool(name="rrep", bufs=r_bufs) as rpool,
            tc.tile_pool(name="yout", bufs=2) as ypool,
            tc.tile_pool(name="acc", bufs=psum_bufs, space="PSUM") as psum,
        ):
            t_s = tpool.tile([128, KS, 128], in_dtype)
            nc.sync.dma_start(t_s[:], t_d[:])
            ka = 14 * CORNER_SPLIT  # 126
            kb = 14 * (KS - CORNER_SPLIT)  # 84
            if loop_order == "corner":
                t2a_s = tpool.tile([ka, 128], in_dtype)
                t2b_s = tpool.tile([kb, 128], in_dtype)
                nc.sync.dma_start(t2a_s[:], t2_d[0:ka, :])
                nc.sync.dma_start(t2b_s[:], t2_d[ka : ka + kb, :])

            def mtile_c_dj(m0, m, kp, x_s, y_s):
                for c0, n in N_TILES:
                    acc = psum.tile([128, 512], F32, tag="acc")
                    for dj in range(KS):
                        nc.tensor.matmul(
                            acc[0:m, 0:n],
                            t_s[0:kp, 0 if same_stationary else dj, 0:m],
                            x_s[0:kp, c0 + dj : c0 + dj + n],
                            start=(dj == 0),
                            stop=(dj == KS - 1),
                        )
                    if evacuate:
                        eng = nc.any if evac_any else nc.vector
                        eng.tensor_scalar_add(
                            y_s[0:m, c0 : c0 + n], acc[0:m, 0:n], bias_val
                        )
                        if y_per_ctile:
                            nc.sync.dma_start(
                                y_d[m0 : m0 + m, c0 : c0 + n],
                                y_s[0:m, c0 : c0 + n],
                            )

            def mtile_dj_c(m0, m, kp, x_s, y_s):
                accs = [
                    psum.tile([128, 512], F32, tag=f"acc{i}", name=f"acc{i}")
                    for i in range(len(N_TILES))
                ]
                for dj in range(KS):
                    for ci, (c0, n) in enumerate(N_TILES):
                        nc.tensor.matmul(
                            accs[ci][0:m, 0:n],
                            t_s[0:kp, dj, 0:m],
                            x_s[0:kp, c0 + dj : c0 + dj + n],
                            start=(dj == 0),
                            stop=(dj == KS - 1),
                        )
                if evacuate:
                    for ci, (c0, n) in enumerate(N_TILES):
                        nc.vector.tensor_scalar_add(
                            y_s[0:m, c0 : c0 + n], accs[ci][0:m, 0:n], bias_val
                        )

            def mtile_corner(m0, x_s, ra, rb, y_s):
                nt = (
                    [(c, min(n_wide, OUT_W - c)) for c in range(0, OUT_W, n_wide)]
                    if n_wide != 512
                    else N_TILES
                )
                for c0, n in nt:
                    acc = psum.tile([128, n_wide], F32, tag="acc")
                    for dj in range(KS):
                        nc.tensor.matmul(
                            acc[:, 0:n],
                            t_s[:, dj, :],
                            x_s[:, c0 + dj : c0 + dj + n],
                            start=(dj == 0),
                            stop=(no_corner_mm and dj == KS - 1),
                        )
                    if no_corner_mm:
                        if evacuate and not psum_dma:
                            nc.vector.tensor_scalar_add(
                                y_s[:, c0 : c0 + n], acc[:, 0:n], bias_val
                            )
                        continue
                    nc.tensor.matmul(
                        acc[:, 0:n],
                        t2a_s[:],
                        ra[0:ka, c0 : c0 + n],
                        start=False,
                        stop=False,
                    )
                    nc.tensor.matmul(
                        acc[:, 0:n],
                        t2b_s[:],
                        rb[0:kb, c0 : c0 + n],
                        start=False,
                        stop=True,
                    )
                    if evacuate:
                        if psum_dma:
                            nc.sync.dma_start(
                                y_d[m0 : m0 + 128, c0 : c0 + n], acc[:, 0:n]
                            )
                        else:
                            eng = nc.any if evac_any else nc.vector
                            eng.tensor_scalar_add(
                                y_s[:, c0 : c0 + n], acc[:, 0:n], bias_val
                            )
                            if y_per_ctile:
                                nc.sync.dma_start(
                                    y_d[m0 : m0 + 128, c0 : c0 + n],
                                    y_s[:, c0 : c0 + n],
                                )

            def mtile_corner_djc(m0, x_s, ra, rb, y_s):
                accs = [
                    psum.tile([128, 512], F32, tag=f"acc{ci}", name=f"acc{ci}")
                    for ci in range(len(N_TILES))
                ]
                for dj in range(KS):
                    for ci, (c0, n) in enumerate(N_TILES):
                        nc.tensor.matmul(
                            accs[ci][:, 0:n],
                            t_s[:, dj, :],
                            x_s[:, c0 + dj : c0 + dj + n],
                            start=(dj == 0),
                            stop=False,
                            skip_group_check=True,
                        )
                for ci, (c0, n) in enumerate(N_TILES):
                    nc.tensor.matmul(
                        accs[ci][:, 0:n],
                        t2a_s[:],
                        ra[0:ka, c0 : c0 + n],
                        start=False,
                        stop=False,
                        skip_group_check=True,
                    )
                for ci, (c0, n) in enumerate(N_TILES):
                    nc.tensor.matmul(
                        accs[ci][:, 0:n],
                        t2b_s[:],
                        rb[0:kb, c0 : c0 + n],
                        start=False,
                        stop=True,
                        skip_group_check=True,
                    )
                    if evacuate:
                        if psum_dma:
                            nc.sync.dma_start(
                                y_d[m0 : m0 + 128, c0 : c0 + n],
                                accs[ci][:, 0:n],
                            )
                        else:
                            nc.vector.tensor_scalar_add(
                                y_s[:, c0 : c0 + n], accs[ci][:, 0:n], bias_val
                            )

            def body_corner():
                for _ in range(repeat):
                    for ti, (m0, m) in enumerate(M_TILES_CORNER):
                        x_s = xpool.tile([128, W], in_dtype, tag="xs")
                        nc.sync.dma_start(x_s[:], x_d[m0 : m0 + 128, :])
                        ra = rpool.tile([128, OUT_W], in_dtype, tag="ra")
                        rb = rpool.tile([128, OUT_W], in_dtype, tag="rb")
                        if no_corner_mm:
                            pass
                        elif r_split > 1:
                            ha = ka // 2
                            hb = kb // 2
                            nc.sync.dma_start(ra[0:ha, :], r_d[ti, 0:ha, :])
                            nc.sync.dma_start(ra[ha:ka, :], r_d[ti, ha:ka, :])
                            nc.sync.dma_start(
                                rb[0:hb, :], r_d[ti, ka : ka + hb, :]
                            )
                            nc.sync.dma_start(
                                rb[hb:kb, :], r_d[ti, ka + hb : ka + kb, :]
                            )
                        else:
                            nc.sync.dma_start(ra[0:ka, :], r_d[ti, 0:ka, :])
                            nc.sync.dma_start(rb[0:kb, :], r_d[ti, ka : ka + kb, :])
                        y_s = (
                            None
                            if psum_dma
                            else ypool.tile([128, OUT_W], y_dt, tag="ys")
                        )
                        if djc:
                            mtile_corner_djc(m0, x_s, ra, rb, y_s)
                        else:
                            mtile_corner(m0, x_s, ra, rb, y_s)
                        if evacuate and not psum_dma and not y_per_ctile:
                            nc.sync.dma_start(y_d[m0 : m0 + 128, :], y_s[:])

            def body():
                if loop_order == "corner":
                    body_corner()
                    return
                for _ in range(repeat):
                    for m0, m in M_TILES:
                        kp = m + KS - 1
                        x_s = xpool.tile([128, W], in_dtype, tag="xs")
                        if split_dma > 1:
                            step = (kp + split_dma - 1) // split_dma
                            for p in range(0, kp, step):
                                pe = min(p + step, kp)
                                nc.sync.dma_start(
                                    x_s[p:pe, :], x_d[m0 + p : m0 + pe, :]
                                )
                        else:
                            nc.sync.dma_start(x_s[0:kp, :], x_d[m0 : m0 + kp, :])
                        y_s = ypool.tile([128, OUT_W], y_dt, tag="ys")
                        if loop_order == "c_dj":
                            mtile_c_dj(m0, m, kp, x_s, y_s)
                        else:
                            mtile_dj_c(m0, m, kp, x_s, y_s)
                        if evacuate and not (loop_order == "c_dj" and y_per_ctile):
                            if split_dma > 1:
                                cstep = (OUT_W + split_dma - 1) // split_dma
                                for c in range(0, OUT_W, cstep):
                                    ce = min(c + cstep, OUT_W)
                                    nc.sync.dma_start(
                                        y_d[m0 : m0 + m, c:ce], y_s[0:m, c:ce]
                                    )
                            else:
                                nc.sync.dma_start(y_d[m0 : m0 + m, :], y_s[0:m, :])

            def body_pure_mm():
                x_s = xpool.tile([128, W], in_dtype, tag="xs")
                nc.sync.dma_start(x_s[:], x_d[0:128, :])

                def inner():
                    mt = M_TILES_CORNER if pure_mm_m128 else M_TILES
                    for _ in range(repeat):
                        for m0, m in mt:
                            kp = 128 if pure_mm_m128 else m + KS - 1
                            if x_redma:
                                x_t = xpool.tile([128, W], in_dtype, tag="xs")
                                nc.sync.dma_start(
                                    x_t[:], x_d[m0 : m0 + 128, :]
                                )
                            else:
                                x_t = x_s
                            for c0, n in N_TILES:
                                acc = psum.tile([128, 512], F32, tag="acc")
                                for dj in range(KS):
                                    nc.tensor.matmul(
                                        acc[0:m, 0:n],
                                        t_s[
                                            0:kp,
                                            0 if same_stationary else dj,
                                            0:m,
                                        ],
                                        x_t[0:kp, c0 + dj : c0 + dj + n],
                                        start=(dj == 0),
                                        stop=(dj == KS - 1),
                                    )

                if loop_repeat > 1:
                    with tc.For_i(0, loop_repeat, 1):
                        inner()
                else:
                    inner()

            if pure_mm:
                body_pure_mm()
            elif loop_repeat > 1:
                with tc.For_i(0, loop_repeat, 1):
                    body()
            else:
                body()
    return nc


def _toeplitz(weight: np.ndarray) -> np.ndarray:
    """T[k, dj, m] = weight[k - m, dj] for 0 <= k - m < 15, else 0."""
    t = np.zeros((128, KS, 128), dtype=np.float32)
    k = np.arange(128)[:, None]
    m = np.arange(128)[None, :]
    d = k - m  # [128, 128]
    mask = (d >= 0) & (d < KS)
    for dj in range(KS):
        col = np.zeros((128, 128), dtype=np.float32)
        col[mask] = weight[d[mask], dj]
        t[:, dj, :] = col
    return t


def _toeplitz_corner(weight: np.ndarray) -> np.ndarray:
    """T2[14*dj + k', m] = weight[128 + k' - m, dj] for m in [114+k', 127]."""
    t2 = np.zeros((14 * KS, 128), dtype=np.float32)
    for dj in range(KS):
        for k_ in range(14):
            m = np.arange(114 + k_, 128)
            t2[14 * dj + k_, m] = weight[128 + k_ - m, dj]
    return t2


def _replicated_seam(x_core: np.ndarray) -> np.ndarray:
    """r[tile, 14*dj' + k', q] = x_core[128*tile + 128 + k', q + dj], with the
    dj >= CORNER_SPLIT blocks packed after the first 14*CORNER_SPLIT rows."""
    r = np.zeros((len(M_TILES_CORNER), 14 * KS, OUT_W), dtype=np.float32)
    for ti, (m0, _) in enumerate(M_TILES_CORNER):
        rows = x_core[m0 + 128 : m0 + 142]  # [14, W]
        for dj in range(KS):
            p0 = (
                14 * dj
                if dj < CORNER_SPLIT
                else 14 * CORNER_SPLIT + 14 * (dj - CORNER_SPLIT)
            )
            r[ti, p0 : p0 + 14, :] = rows[:, dj : dj + OUT_W]
    return r


def _prepare_inputs(x: np.ndarray, weight: np.ndarray, np_dtype=None):
    if np_dtype is None:
        import ml_dtypes

        np_dtype = ml_dtypes.bfloat16
    x_pad = np.zeros((N_CORES * ROWS_PER_CORE + KS - 1, W), dtype=np.float32)
    x_pad[:H] = x
    t = _toeplitz(weight).astype(np_dtype)
    t2 = _toeplitz_corner(weight).astype(np_dtype)
    in_maps = []
    for c in range(N_CORES):
        r0 = c * ROWS_PER_CORE
        xc = np.ascontiguousarray(x_pad[r0 : r0 + IN_ROWS])
        in_maps.append(
            {
                "x": xc.astype(np_dtype),
                "t": t,
                "t2": t2,
                "r": _replicated_seam(xc).astype(np_dtype),
            }
        )
    return in_maps


DEFAULT_BUILD = {
    "loop_order": "corner",
    "psum_bufs": 6,
    "in_dtype": BF16,
    "y_bf16": True,
    "y_per_ctile": True,
}


def run(x: np.ndarray, weight: np.ndarray, bias: np.ndarray, repeat: int = 1, **kw):
    bkw = {**DEFAULT_BUILD, **kw}
    nc = build_program(float(bias[0]), repeat=repeat, **bkw)
    np_dtype = np.float32 if bkw.get("in_dtype") is F32R else None
    in_maps = _prepare_inputs(x, weight, np_dtype)
    res = run_bass_kernel_spmd(nc, in_maps, list(range(N_CORES)))
    full = np.concatenate([res.results[c]["y"] for c in range(N_CORES)], axis=0)
    return np.ascontiguousarray(full[:OUT_H]).astype(np.float32)


def kernel(x: np.ndarray, weight: np.ndarray, bias: np.ndarray) -> np.ndarray:
    return run(x, weight, bias, repeat=1)



# revision 3
# speedup vs baseline: 2.0088x; 2.0088x over previous
"""TRN2 Bass kernel: 4096x4096 fp32 'valid' cross-correlation with a 15x15
kernel (+ scalar bias), sharded row-wise across 8 NeuronCores.

Formulation (per core, per output row-tile of 128 rows, "corner" scheme):
  out[i, j] = sum_dj sum_di w[di, dj] * x[i + di, j + dj]
For each kernel column dj, the contraction over di is a banded-Toeplitz
matmul over input rows: stationary lhsT[k, m] = w[k - m, dj] (15-diagonal
band), moving rhs = x rows with a free-dim column offset of dj. The 15
dj-matmuls accumulate in one PSUM bank; the 14 seam rows per tile are
completed by two extra matmuls over replicated shifted copies (r) of the
14 rows below the tile's K-block, so each 128-row tile costs exactly
ceil(142*15/128) = 17 matmul streams — the PE coverage floor.

Matmuls run in bfloat16 (stationary and moving): 1 col/cycle on the PE
with the per-matmul weight load hidden (fast weight load path; fp32r
self-loads serialize ~90ns/matmul instead), at ~2.4e-3 relative error
(gate is 2e-2). The output is evacuated PSUM->SBUF as bf16 (halves y DMA
traffic) and the host casts back to fp32.

Each core gets 512 padded output rows (input slice of 526 rows including
the 14-row halo); the host pads x to 4110 rows and drops the 14 garbage
output rows at the end.
"""

import os
import sys

for _p in ("/opt/trn_rl_repo",):
    if os.path.isdir(_p) and _p not in sys.path:
        sys.path.insert(0, _p)

import json

import numpy as np

import concourse.bass as bass
import concourse.tile as tile
from concourse import mybir
from concourse.bass_utils import run_bass_kernel_spmd

# ---------------------------------------------------------------------------
# Workaround: the installed walrus_driver rejects instructions carrying more
# than one sync wait ("Too many sync wait commands"). TileContext's kernel-tail
# drain carries one wait per outstanding semaphore. Splitting each extra wait
# into its own single-wait EventSemaphore on the same engine right before the
# original instruction is semantically identical (same-engine program order;
# semaphores are monotone).
# ---------------------------------------------------------------------------
_orig_to_json_bytes = bass.Bass.to_json_bytes


def _split_multi_waits(bir: dict) -> dict:
    n = 0
    for fn in bir.get("functions", []):
        for bb in fn.get("blocks", []):
            insts = bb.get("instructions")
            if not insts:
                continue
            out = []
            for inst in insts:
                si = inst.get("sync_info") or {}
                waits = si.get("on_wait") or []
                if len(waits) > 1:
                    for w in waits[:-1]:
                        n += 1
                        out.append(
                            {
                                "debug": inst.get("debug", 0),
                                "engine": inst["engine"],
                                "ins": [],
                                "name": f"{inst['name']}-waitsplit{n}",
                                "opcode": "EventSemaphore",
                                "outs": [],
                                "sync_info": {"on_update": [], "on_wait": [w]},
                            }
                        )
                    si["on_wait"] = [waits[-1]]
                out.append(inst)
            bb["instructions"] = out
    return bir


def _dedup_ldweights(bir: dict) -> dict:
    """Delete PE Ldweights whose (tile_position, tile_size, weights AP) is
    identical to the weights already resident at that tile position (weights
    persist across matmuls until another Ldweights targets the position).
    Amortizes the ~27ns serialized weight load across matmuls that share a
    stationary. Waits/updates on a deleted Ldweights move to a standalone
    EventSemaphore."""
    n = 0
    for fn in bir.get("functions", []):
        for bb in fn.get("blocks", []):
            insts = bb.get("instructions")
            if not insts:
                continue
            state: dict = {}
            out = []
            for inst in insts:
                if inst.get("opcode") != "Ldweights":
                    out.append(inst)
                    continue
                pos = tuple(inst.get("tile_position") or (0, 0))
                sig = json.dumps(
                    [inst.get("ins"), inst.get("tile_size")], sort_keys=True
                )
                if state.get(pos) == sig:
                    n += 1
                    si = inst.get("sync_info") or {}
                    waits = si.get("on_wait") or []
                    ups = si.get("on_update") or []
                    if waits or ups:
                        out.append(
                            {
                                "debug": inst.get("debug", 0),
                                "engine": inst["engine"],
                                "ins": [],
                                "name": f"{inst['name']}-ldwdedup{n}",
                                "opcode": "EventSemaphore",
                                "outs": [],
                                "sync_info": {"on_update": ups, "on_wait": waits},
                            }
                        )
                    continue
                state[pos] = sig
                out.append(inst)
            bb["instructions"] = out
    return bir


_DEDUP_LDW = [False]


def _patched_to_json_bytes(self, *args, **kwargs):
    raw = _orig_to_json_bytes(self, *args, **kwargs)
    bir = _split_multi_waits(json.loads(raw))
    if _DEDUP_LDW[0]:
        bir = _dedup_ldweights(bir)
    return json.dumps(bir).encode()


bass.Bass.to_json_bytes = _patched_to_json_bytes

# ---------------------------------------------------------------------------

H = W = 4096
KS = 15
OUT_H = H - KS + 1  # 4082
OUT_W = W - KS + 1  # 4082
N_CORES = 8
ROWS_PER_CORE = 512  # padded output rows per core (8 * 512 = 4096 >= 4082)
IN_ROWS = ROWS_PER_CORE + KS - 1  # 526

# Output row-tiles per core: M <= 114 so the band (M + 14) fits in K <= 128.
M_TILES = [(0, 114), (114, 114), (228, 114), (342, 114), (456, 56)]
# Corner scheme: 4 tiles of 128 rows; the 14 seam rows per tile are completed
# by two extra matmuls contracting (dj, k') pairs over replicated shifted
# copies of the 14 rows below the tile's K-block.
M_TILES_CORNER = [(0, 128), (128, 128), (256, 128), (384, 128)]
CORNER_SPLIT = 9  # dj 0..8 -> corner MM a (K=126), dj 9..14 -> MM b (K=84)
# Output column tiles: N <= 512 (one fp32 PSUM bank).
N_TILES = [(c, min(512, OUT_W - c)) for c in range(0, OUT_W, 512)]

F32R = mybir.dt.float32r
F32 = mybir.dt.float32
BF16 = mybir.dt.bfloat16


def build_program(
    bias_val: float,
    repeat: int = 1,
    loop_repeat: int = 1,
    loop_order: str = "c_dj",
    evacuate: bool = True,
    psum_bufs: int = 4,
    same_stationary: bool = False,
    pure_mm: bool = False,
    x_bufs: int = 2,
    y_per_ctile: bool = False,
    split_dma: int = 1,
    evac_any: bool = False,
    r_split: int = 1,
    in_dtype=F32R,
    psum_dma: bool = False,
    djc: bool = False,
    no_corner_mm: bool = False,
    pure_mm_m128: bool = False,
    x_redma: bool = False,
    y_bf16: bool = False,
    n_wide: int = 512,
    r_bufs: int = 2,
    dedup_ldw: bool = False,
) -> bass.Bass:
    _DEDUP_LDW[0] = dedup_ldw
    nc = bass.Bass()
    x_d = nc.dram_tensor("x", [IN_ROWS, W], in_dtype, kind="ExternalInput")
    t_d = nc.dram_tensor("t", [128, KS, 128], in_dtype, kind="ExternalInput")
    t2_d = nc.dram_tensor("t2", [14 * KS, 128], in_dtype, kind="ExternalInput")
    r_d = nc.dram_tensor(
        "r", [len(M_TILES_CORNER), 14 * KS, OUT_W], in_dtype, kind="ExternalInput"
    )
    y_dt = BF16 if y_bf16 else F32
    y_d = nc.dram_tensor("y", [ROWS_PER_CORE, OUT_W], y_dt, kind="ExternalOutput")

    with tile.TileContext(nc) as tc:
        with (
            tc.tile_pool(name="tconst", bufs=1) as tpool,
            tc.tile_pool(name="xin", bufs=x_bufs) as xpool,
            tc.tile_pool(name="rrep", bufs=r_bufs) as rpool,
            tc.tile_pool(name="yout", bufs=2) as ypool,
            tc.tile_pool(name="acc", bufs=psum_bufs, space="PSUM") as psum,
        ):
            t_s = tpool.tile([128, KS, 128], in_dtype)
            nc.sync.dma_start(t_s[:], t_d[:])
            ka = 14 * CORNER_SPLIT  # 126
            kb = 14 * (KS - CORNER_SPLIT)  # 84
            if loop_order == "corner":
                t2a_s = tpool.tile([ka, 128], in_dtype)
                t2b_s = tpool.tile([kb, 128], in_dtype)
                nc.sync.dma_start(t2a_s[:], t2_d[0:ka, :])
                nc.sync.dma_start(t2b_s[:], t2_d[ka : ka + kb, :])

            def mtile_c_dj(m0, m, kp, x_s, y_s):
                for c0, n in N_TILES:
                    acc = psum.tile([128, 512], F32, tag="acc")
                    for dj in range(KS):
                        nc.tensor.matmul(
                            acc[0:m, 0:n],
                            t_s[0:kp, 0 if same_stationary else dj, 0:m],
                            x_s[0:kp, c0 + dj : c0 + dj + n],
                            start=(dj == 0),
                            stop=(dj == KS - 1),
                        )
                    if evacuate:
                        eng = nc.any if evac_any else nc.vector
                        eng.tensor_scalar_add(
                            y_s[0:m, c0 : c0 + n], acc[0:m, 0:n], bias_val
                        )
                        if y_per_ctile:
                            nc.sync.dma_start(
                                y_d[m0 : m0 + m, c0 : c0 + n],
                                y_s[0:m, c0 : c0 + n],
                            )

            def mtile_dj_c(m0, m, kp, x_s, y_s):
                accs = [
                    psum.tile([128, 512], F32, tag=f"acc{i}", name=f"acc{i}")
                    for i in range(len(N_TILES))
                ]
                for dj in range(KS):
                    for ci, (c0, n) in enumerate(N_TILES):
                        nc.tensor.matmul(
                            accs[ci][0:m, 0:n],
                            t_s[0:kp, dj, 0:m],
                            x_s[0:kp, c0 + dj : c0 + dj + n],
                            start=(dj == 0),
                            stop=(dj == KS - 1),
                        )
                if evacuate:
                    for ci, (c0, n) in enumerate(N_TILES):
                        nc.vector.tensor_scalar_add(
                            y_s[0:m, c0 : c0 + n], accs[ci][0:m, 0:n], bias_val
                        )

            def mtile_corner(m0, x_s, ra, rb, y_s):
                nt = (
                    [(c, min(n_wide, OUT_W - c)) for c in range(0, OUT_W, n_wide)]
                    if n_wide != 512
                    else N_TILES
                )
                for c0, n in nt:
                    acc = psum.tile([128, n_wide], F32, tag="acc")
                    for dj in range(KS):
                        nc.tensor.matmul(
                            acc[:, 0:n],
                            t_s[:, dj, :],
                            x_s[:, c0 + dj : c0 + dj + n],
                            start=(dj == 0),
                            stop=(no_corner_mm and dj == KS - 1),
                        )
                    if no_corner_mm:
                        if evacuate and not psum_dma:
                            nc.vector.tensor_scalar_add(
                                y_s[:, c0 : c0 + n], acc[:, 0:n], bias_val
                            )
                        continue
                    nc.tensor.matmul(
                        acc[:, 0:n],
                        t2a_s[:],
                        ra[0:ka, c0 : c0 + n],
                        start=False,
                        stop=False,
                    )
                    nc.tensor.matmul(
                        acc[:, 0:n],
                        t2b_s[:],
                        rb[0:kb, c0 : c0 + n],
                        start=False,
                        stop=True,
                    )
                    if evacuate:
                        if psum_dma:
                            nc.sync.dma_start(
                                y_d[m0 : m0 + 128, c0 : c0 + n], acc[:, 0:n]
                            )
                        else:
                            eng = nc.any if evac_any else nc.vector
                            eng.tensor_scalar_add(
                                y_s[:, c0 : c0 + n], acc[:, 0:n], bias_val
                            )
                            if y_per_ctile:
                                nc.sync.dma_start(
                                    y_d[m0 : m0 + 128, c0 : c0 + n],
                                    y_s[:, c0 : c0 + n],
                                )

            def mtile_corner_djc(m0, x_s, ra, rb, y_s):
                accs = [
                    psum.tile([128, 512], F32, tag=f"acc{ci}", name=f"acc{ci}")
                    for ci in range(len(N_TILES))
                ]
                for dj in range(KS):
                    for ci, (c0, n) in enumerate(N_TILES):
                        nc.tensor.matmul(
                            accs[ci][:, 0:n],
                            t_s[:, dj, :],
                            x_s[:, c0 + dj : c0 + dj + n],
                            start=(dj == 0),
                            stop=False,
                            skip_group_check=True,
                        )
                for ci, (c0, n) in enumerate(N_TILES):
                    nc.tensor.matmul(
                        accs[ci][:, 0:n],
                        t2a_s[:],
                        ra[0:ka, c0 : c0 + n],
                        start=False,
                        stop=False,
                        skip_group_check=True,
                    )
                for ci, (c0, n) in enumerate(N_TILES):
                    nc.tensor.matmul(
                        accs[ci][:, 0:n],
                        t2b_s[:],
                        rb[0:kb, c0 : c0 + n],
                        start=False,
                        stop=True,
                        skip_group_check=True,
                    )
                    if evacuate:
                        if psum_dma:
                            nc.sync.dma_start(
                                y_d[m0 : m0 + 128, c0 : c0 + n],
                                accs[ci][:, 0:n],
                            )
                        else:
                            nc.vector.tensor_scalar_add(
                                y_s[:, c0 : c0 + n], accs[ci][:, 0:n], bias_val
                            )

            def body_corner():
                for _ in range(repeat):
                    for ti, (m0, m) in enumerate(M_TILES_CORNER):
                        x_s = xpool.tile([128, W], in_dtype, tag="xs")
                        nc.sync.dma_start(x_s[:], x_d[m0 : m0 + 128, :])
                        ra = rpool.tile([128, OUT_W], in_dtype, tag="ra")
                        rb = rpool.tile([128, OUT_W], in_dtype, tag="rb")
                        if no_corner_mm:
                            pass
                        elif r_split > 1:
                            ha = ka // 2
                            hb = kb // 2
                            nc.sync.dma_start(ra[0:ha, :], r_d[ti, 0:ha, :])
                            nc.sync.dma_start(ra[ha:ka, :], r_d[ti, ha:ka, :])
                            nc.sync.dma_start(
                                rb[0:hb, :], r_d[ti, ka : ka + hb, :]
                            )
                            nc.sync.dma_start(
                                rb[hb:kb, :], r_d[ti, ka + hb : ka + kb, :]
                            )
                        else:
                            nc.sync.dma_start(ra[0:ka, :], r_d[ti, 0:ka, :])
                            nc.sync.dma_start(rb[0:kb, :], r_d[ti, ka : ka + kb, :])
                        y_s = (
                            None
                            if psum_dma
                            else ypool.tile([128, OUT_W], y_dt, tag="ys")
                        )
                        if djc:
                            mtile_corner_djc(m0, x_s, ra, rb, y_s)
                        else:
                            mtile_corner(m0, x_s, ra, rb, y_s)
                        if evacuate and not psum_dma and not y_per_ctile:
                            nc.sync.dma_start(y_d[m0 : m0 + 128, :], y_s[:])

            def body():
                if loop_order == "corner":
                    body_corner()
                    return
                for _ in range(repeat):
                    for m0, m in M_TILES:
                        kp = m + KS - 1
                        x_s = xpool.tile([128, W], in_dtype, tag="xs")
                        if split_dma > 1:
                            step = (kp + split_dma - 1) // split_dma
                            for p in range(0, kp, step):
                                pe = min(p + step, kp)
                                nc.sync.dma_start(
                                    x_s[p:pe, :], x_d[m0 + p : m0 + pe, :]
                                )
                        else:
                            nc.sync.dma_start(x_s[0:kp, :], x_d[m0 : m0 + kp, :])
                        y_s = ypool.tile([128, OUT_W], y_dt, tag="ys")
                        if loop_order == "c_dj":
                            mtile_c_dj(m0, m, kp, x_s, y_s)
                        else:
                            mtile_dj_c(m0, m, kp, x_s, y_s)
                        if evacuate and not (loop_order == "c_dj" and y_per_ctile):
                            if split_dma > 1:
                                cstep = (OUT_W + split_dma - 1) // split_dma
                                for c in range(0, OUT_W, cstep):
                                    ce = min(c + cstep, OUT_W)
                                    nc.sync.dma_start(
                                        y_d[m0 : m0 + m, c:ce], y_s[0:m, c:ce]
                                    )
                            else:
                                nc.sync.dma_start(y_d[m0 : m0 + m, :], y_s[0:m, :])

            def body_pure_mm():
                x_s = xpool.tile([128, W], in_dtype, tag="xs")
                nc.sync.dma_start(x_s[:], x_d[0:128, :])

                def inner():
                    mt = M_TILES_CORNER if pure_mm_m128 else M_TILES
                    for _ in range(repeat):
                        for m0, m in mt:
                            kp = 128 if pure_mm_m128 else m + KS - 1
                            if x_redma:
                                x_t = xpool.tile([128, W], in_dtype, tag="xs")
                                nc.sync.dma_start(
                                    x_t[:], x_d[m0 : m0 + 128, :]
                                )
                            else:
                                x_t = x_s
                            for c0, n in N_TILES:
                                acc = psum.tile([128, 512], F32, tag="acc")
                                for dj in range(KS):
                                    nc.tensor.matmul(
                                        acc[0:m, 0:n],
                                        t_s[
                                            0:kp,
                                            0 if same_stationary else dj,
                                            0:m,
                                        ],
                                        x_t[0:kp, c0 + dj : c0 + dj + n],
                                        start=(dj == 0),
                                        stop=(dj == KS - 1),
                                    )

                if loop_repeat > 1:
                    with tc.For_i(0, loop_repeat, 1):
                        inner()
                else:
                    inner()

            if pure_mm:
                body_pure_mm()
            elif loop_repeat > 1:
                with tc.For_i(0, loop_repeat, 1):
                    body()
            else:
                body()
    return nc


def _toeplitz(weight: np.ndarray) -> np.ndarray:
    """T[k, dj, m] = weight[k - m, dj] for 0 <= k - m < 15, else 0."""
    t = np.zeros((128, KS, 128), dtype=np.float32)
    k = np.arange(128)[:, None]
    m = np.arange(128)[None, :]
    d = k - m  # [128, 128]
    mask = (d >= 0) & (d < KS)
    for dj in range(KS):
        col = np.zeros((128, 128), dtype=np.float32)
        col[mask] = weight[d[mask], dj]
        t[:, dj, :] = col
    return t


def _toeplitz_corner(weight: np.ndarray) -> np.ndarray:
    """T2[14*dj + k', m] = weight[128 + k' - m, dj] for m in [114+k', 127]."""
    t2 = np.zeros((14 * KS, 128), dtype=np.float32)
    for dj in range(KS):
        for k_ in range(14):
            m = np.arange(114 + k_, 128)
            t2[14 * dj + k_, m] = weight[128 + k_ - m, dj]
    return t2


def _replicated_seam(x_core: np.ndarray) -> np.ndarray:
    """r[tile, 14*dj' + k', q] = x_core[128*tile + 128 + k', q + dj], with the
    dj >= CORNER_SPLIT blocks packed after the first 14*CORNER_SPLIT rows."""
    r = np.zeros((len(M_TILES_CORNER), 14 * KS, OUT_W), dtype=np.float32)
    for ti, (m0, _) in enumerate(M_TILES_CORNER):
        rows = x_core[m0 + 128 : m0 + 142]  # [14, W]
        for dj in range(KS):
            p0 = (
                14 * dj
                if dj < CORNER_SPLIT
                else 14 * CORNER_SPLIT + 14 * (dj - CORNER_SPLIT)
            )
            r[ti, p0 : p0 + 14, :] = rows[:, dj : dj + OUT_W]
    return r


def _prepare_inputs(x: np.ndarray, weight: np.ndarray, np_dtype=None):
    if np_dtype is None:
        import ml_dtypes

        np_dtype = ml_dtypes.bfloat16
    x_pad = np.zeros((N_CORES * ROWS_PER_CORE + KS - 1, W), dtype=np.float32)
    x_pad[:H] = x
    t = _toeplitz(weight).astype(np_dtype)
    t2 = _toeplitz_corner(weight).astype(np_dtype)
    in_maps = []
    for c in range(N_CORES):
        r0 = c * ROWS_PER_CORE
        xc = np.ascontiguousarray(x_pad[r0 : r0 + IN_ROWS])
        in_maps.append(
            {
                "x": xc.astype(np_dtype),
                "t": t,
                "t2": t2,
                "r": _replicated_seam(xc).astype(np_dtype),
            }
        )
    return in_maps


DEFAULT_BUILD = {
    "loop_order": "corner",
    "psum_bufs": 6,
    "in_dtype": BF16,
    "y_bf16": True,
    "y_per_ctile": True,
}


def run(x: np.ndarray, weight: np.ndarray, bias: np.ndarray, repeat: int = 1, **kw):
    bkw = {**DEFAULT_BUILD, **kw}
    nc = build_program(float(bias[0]), repeat=repeat, **bkw)
    np_dtype = np.float32 if bkw.get("in_dtype") is F32R else None
    in_maps = _prepare_inputs(x, weight, np_dtype)
    res = run_bass_kernel_spmd(nc, in_maps, list(range(N_CORES)))
    full = np.concatenate([res.results[c]["y"] for c in range(N_CORES)], axis=0)
    return np.ascontiguousarray(full[:OUT_H]).astype(np.float32)


def kernel(x: np.ndarray, weight: np.ndarray, bias: np.ndarray) -> np.ndarray:
    return run(x, weight, bias, repeat=1)

